# revision 1
# baseline (speedup 1.0000x reference)
"""Trainium2 Bass kernel for nn_AttnModel (BiAttn x3 + tiny FC + batch-softmax tile).

Contract: kernel(**inputs) takes the FULL inputs (a_emb/v_emb/l_emb [32,1024,32],
fc1_w [64,64], fc1_b [64], fc2_w [1,64]) and returns the FULL output [32,1024,64].

Strategy (8 NeuronCores, data-parallel over batch, 4 batches/core):
  Per (batch, pair) "unit" (pairs: (a,v), (a,l), (v,l)) the reference only uses
  row 0 of each BiAttn output, which needs:
    S = f @ g^T  [1024,1024]      (PE, K=32 row-tiled, one unit per 32-row tile)
    E = exp(S)                    (ScalarE, fused row-sum via accum_out)
    colsum c_j = sum_i E_ij       (PE ones-matmul, M=1, col-tiled 4 units)
    w1_j = E_0j / c_j ; o1 = w1 @ g
    w2_i = E_i0 / rowsum_i ; o2 = w2 @ f
    Bi_row = [o1*f_0, o2*g_0]  (64)
  Tiny FC -> logits Ci [4,3]; exp(Ci) AllGathered across the 8 cores for the
  batch-dim softmax; output row per batch broadcast-written as [1024, 64].
"""
import numpy as np
import ml_dtypes

import concourse.bass as bass
import concourse.bacc as bacc
import concourse.tile as tile
import concourse.mybir as mybir
from concourse.bass_utils import run_bass_kernel_spmd
from concourse.tile_rust import add_dep_helper

F32 = mybir.dt.float32
F32R = mybir.dt.float32r
BF16 = mybir.dt.bfloat16
AF = mybir.ActivationFunctionType

B, U, D = 32, 1024, 32
NCORES = 8
BPC = B // NCORES          # batches per core = 4
NU = 3 * BPC               # units per core = 12
NPACK = NU // 4            # packs of 4 units = 3
NCH = U // 128             # i-chunks = 8
PAIRS = [(0, 1), (0, 2), (1, 2)]  # (f,g) emb indices for pair k; 0=a 1=v 2=l

_DEBUG = False
import os as _os
S_F32R = _os.environ.get("S_F32R", "0") == "1"


def build_program(repeat=1):
    nc = bacc.Bacc("TRN2", target_bir_lowering=False, debug=False, num_devices=NCORES)

    eblob = nc.dram_tensor("eblob", [NPACK, 128, 8192], BF16, kind="ExternalInput")
    nblob = nc.dram_tensor("nblob", [NPACK, 128, 2048], F32, kind="ExternalInput")
    f0g0 = nc.dram_tensor("f0g0", [NPACK, 128, 64], F32, kind="ExternalInput")
    fc1T = nc.dram_tensor("fc1T", [64, 64], F32, kind="ExternalInput")
    fc1b = nc.dram_tensor("fc1b", [NU, 64], F32, kind="ExternalInput")
    fc2T = nc.dram_tensor("fc2T", [64, 1], F32, kind="ExternalInput")
    out = nc.dram_tensor("out", [BPC, U, 2 * D], F32, kind="ExternalOutput")
    if _DEBUG:
        dbg_bi = nc.dram_tensor("dbg_bi", [NU, 64], F32, kind="ExternalOutput")
        dbg_eci = nc.dram_tensor("dbg_eci", [NU, 1], F32, kind="ExternalOutput")
        dbg_rows = nc.dram_tensor("dbg_rows", [BPC, 64], F32, kind="ExternalOutput")

    ident_np = np.eye(128, dtype=np.float32)
    sel_np = np.zeros((NU, BPC), np.float32)
    for r in range(NU):
        sel_np[r, r // 3] = 1.0
    oh_np = np.zeros((BPC, BPC * 128), np.float32)
    for b in range(BPC):
        oh_np[b, 128 * b:128 * (b + 1)] = 1.0

    with tile.TileContext(nc) as tc:
        from contextlib import ExitStack
        ctx = ExitStack()
        consts = ctx.enter_context(tc.tile_pool(name="consts", bufs=1))
        bigp = ctx.enter_context(tc.tile_pool(name="big", bufs=1))
        epool = ctx.enter_context(tc.tile_pool(name="epool", bufs=1))
        packp = ctx.enter_context(tc.tile_pool(name="packp", bufs=3))
        tailp = ctx.enter_context(tc.tile_pool(name="tailp", bufs=1))
        dramp = ctx.enter_context(tc.tile_pool(name="dramp", bufs=1, space="DRAM"))

        sps = ctx.enter_context(tc.tile_pool(name="sps", bufs=1, space="PSUM"))
        csps = ctx.enter_context(tc.tile_pool(name="csps", bufs=1, space="PSUM"))
        tpps = ctx.enter_context(tc.tile_pool(name="tpps", bufs=1, space="PSUM"))
        ops = ctx.enter_context(tc.tile_pool(name="ops", bufs=1, space="PSUM"))

        # ---------------- first-needed input: pack 0, units 0-1 ----------------
        ebA, ebB = [], []
        d_ebA, d_ebB = [], []
        t_e0 = bigp.tile([128, 4096], BF16, tag="ebA0", name="ebA_0")
        d_ebA.append(nc.sync.dma_start(t_e0[:], eblob[0, :, 0:4096]))
        ebA.append(t_e0)

        # ---------------- constants ----------------
        ident = consts.tile([128, 128], F32)
        d_ident = nc.sync.dma_start(ident[:], nc.inline_tensor(ident_np, name="c_ident")[:, :])
        ones = consts.tile([128, 1], F32)
        d_ones = nc.sync.dma_start(ones[:], nc.inline_tensor(np.ones((128, 1), np.float32), name="c_ones")[:, :])
        ones_r = consts.tile([128, 1], F32)
        d_ones_r = nc.sync.dma_start(ones_r[:].bitcast(F32R), nc.inline_tensor(np.ones((128, 1), np.float32), name="c_ones_r")[:, :].bitcast(F32R))
        selT = consts.tile([NU, BPC], F32)
        d_sel = nc.sync.dma_start(selT[:], nc.inline_tensor(sel_np, name="c_sel")[:, :])
        oh = consts.tile([BPC, BPC * 128], F32)
        d_oh = nc.sync.dma_start(oh[:], nc.inline_tensor(oh_np, name="c_oh")[:, :])
        fc1T_sb = consts.tile([64, 64], F32)
        d_fc1 = nc.sync.dma_start(fc1T_sb[:], fc1T[:, :])
        fc1b_sb = consts.tile([NU, 64], F32)
        d_fc1b = nc.sync.dma_start(fc1b_sb[:], fc1b[:, :])
        fc2T_sb = consts.tile([64, 1], F32)
        d_fc2 = nc.sync.dma_start(fc2T_sb[:], fc2T[:, :])
        ksel_np = np.zeros((3, NU), np.float32)
        for r in range(NU):
            ksel_np[r % 3, r] = 1.0
        ksel3 = consts.tile([3, NU], F32)
        d_ksel = nc.sync.dma_start(ksel3[:], nc.inline_tensor(ksel_np, name="c_ksel")[:, :])

        # ---------------- resident inputs ----------------
        eb, nb, fg = [], [], []
        d_eb, d_nb, d_fg = [], [], []
        for p in range(NPACK):
            if p > 0:
                t_eA = bigp.tile([128, 4096], BF16, tag=f"ebA{p}", name=f"ebA_{p}")
                d_ebA.append(nc.sync.dma_start(t_eA[:], eblob[p, :, 0:4096]))
                ebA.append(t_eA)
            t_eB = bigp.tile([128, 4096], BF16, tag=f"ebB{p}", name=f"ebB_{p}")
            d_ebB.append(nc.sync.dma_start(t_eB[:], eblob[p, :, 4096:8192]))
            ebB.append(t_eB)
            t_n = bigp.tile([128, 2048], F32, tag="nb", bufs=2, name=f"nb_{p}")
            d_nb.append(nc.sync.dma_start(t_n[:], nblob[p, :, :]))
            nb.append(t_n)
            t_f = bigp.tile([128, 64], F32, tag=f"fg{p}")
            d_fg.append(nc.sync.dma_start(t_f[:], f0g0[p, :, :]))
            fg.append(t_f)

        def guard(eng, deps):
            deps = [d for d in deps if d is not None]
            if not deps:
                return None
            n = eng.nop(nofuse=True)
            for d in deps:
                add_dep_helper(n.ins, d.ins, sync=True, reason="wait-carrier")
            return n

        def pin(inst, g):
            if g is not None:
                add_dep_helper(inst.ins, g.ins, sync=False, reason="order")

        biT_sb = tailp.tile([64, NU], F32)

        # ---------------- main loop: one unit at a time, full-unit E ----------------
        SR = F32R if S_F32R else F32
        def maybe_r(ap):
            return ap.bitcast(F32R) if S_F32R else ap

        e_readers = {}   # unit -> DVE reader insts of its E tile
        packdata = {}
        pending_tails = []
        for rep, p in [(r_, p_) for r_ in range(repeat) for p_ in range(NPACK)]:
            e0cs = packp.tile([128, 1024], F32, tag="e0cs", name=f"e0cs_{rep}_{p}")
            s0row = packp.tile([128, 1024], F32, tag="s0row", name=f"s0row_{rep}_{p}")
            csrows = packp.tile([128, 1024], F32, tag="csrows", name=f"csrows_{rep}_{p}")
            nc.gpsimd.memset(s0row[:], 0.0)
            packdata[(rep, p)] = (e0cs, csrows, [], s0row)
            for pair in range(2):
                tpair = (2 * pair, 2 * pair + 1)
                ud = {}
                for t in tpair:
                    u = 4 * (p + NPACK * rep) + t
                    E = epool.tile([128, NCH * 1024], F32, tag=f"E{u % 2}", name=f"E_{u}")
                    rs = packp.tile([128, NCH], F32, tag=f"rs{t}", name=f"rs_{u}")
                    scol0 = packp.tile([128, NCH], F32, tag=f"scol0{t}", name=f"scol0_{u}")
                    g0 = guard(nc.tensor, [d_ebA[p] if t < 2 else d_ebB[p]] if t % 2 == 0 else [])
                    ge = guard(nc.scalar, e_readers.get(u - 2, []))
                    ud[t] = (u, E, rs, scol0, [], g0, ge)
                # chunk-major over the pair: row tiles 32*t0 / 32*t1 stream concurrently
                for c in range(NCH):
                    for t in tpair:
                        u, E, rs, scol0, s_readers, g0, ge = ud[t]
                        S_ps = sps.tile([128, 1024], F32, tag=f"S{u % 2}", name=f"S_{u}_{c}")
                        rb = 32 * (t % 2)
                        eslc = ebA[p] if t < 2 else ebB[p]
                        fhi = eslc[rb:rb + 32, 128 * c:128 * (c + 1)]
                        flo = eslc[rb:rb + 32, 2048 + 128 * c:2048 + 128 * (c + 1)]
                        for h in range(2):
                            ghi = eslc[rb:rb + 32, 1024 + 512 * h:1024 + 512 * (h + 1)]
                            glo = eslc[rb:rb + 32, 3072 + 512 * h:3072 + 512 * (h + 1)]
                            out_h = S_ps[:, 512 * h:512 * (h + 1)]
                            mm = nc.tensor.matmul(out_h, fhi, ghi, start=True, stop=False,
                                                  tile_position=(rb, 0))
                            nc.tensor.matmul(out_h, fhi, glo, start=False, stop=False,
                                             tile_position=(rb, 0))
                            nc.tensor.matmul(out_h, flo, ghi, start=False, stop=True,
                                             tile_position=(rb, 0))
                            if c == 0 and h == 0:
                                pin(mm, g0)
                        act = nc.scalar.activation(
                            E[:, 1024 * c:1024 * (c + 1)].bitcast(F32R), S_ps[:], AF.Exp,
                            accum_out=rs[:, c:c + 1])
                        if c == 0:
                            pin(act, ge)
                            rsp = nc.vector.tensor_copy(s0row[32 * t:32 * t + 1, :], S_ps[0:1, :])
                        csp = nc.vector.tensor_copy(scol0[:, c:c + 1], S_ps[:, 0:1])
                        s_readers.extend([rsp if c == 0 else None, csp])
                # pair tail: colsums + w2 + o2 per unit
                for t in tpair:
                    u, E, rs, scol0, s_readers, g0, ge = ud[t]
                    cs_ps = csps.tile([1, 1024], F32, tag="cs", name=f"cs_{u}")
                    for c in range(NCH):
                        for h in range(2):
                            nc.tensor.matmul(
                                cs_ps[0:1, 512 * h:512 * (h + 1)],
                                ones_r[:, :].bitcast(F32R),
                                E[:, 1024 * c + 512 * h:1024 * c + 512 * (h + 1)].bitcast(F32R),
                                start=(c == 0), stop=(c == NCH - 1),
                            )
                    r_cs = nc.vector.tensor_copy(csrows[32 * t:32 * t + 1, :], cs_ps[0:1, :])
                    ec0 = packp.tile([128, NCH], F32, tag=f"ec0{t}", name=f"ec0_{u}")
                    nc.scalar.activation(ec0[:], scol0[:], AF.Exp)
                    e_readers[u] = [r_cs]
                    rsr = packp.tile([128, NCH], F32, tag=f"rsr{t}", name=f"rsr_{u}")
                    nc.vector.reciprocal(rsr[:], rs[:])
                    w2 = packp.tile([128, NCH], F32, tag=f"w2{t}", name=f"w2_{u}")
                    nc.vector.tensor_mul(w2[:], ec0[:], rsr[:])
                    o_ps = packdata[(rep, p)][2]
                    if t == 0:
                        o_ps.append(ops.tile([128, 64], F32, tag="o", name=f"o_ps_{rep}_{p}"))
                    for c in range(NCH):
                        nc.tensor.matmul(
                            o_ps[0][32 * t:32 * t + 1, 32:64],
                            w2[:, c:c + 1],
                            nb[p][:, 512 * t + 32 * c:512 * t + 32 * (c + 1)],
                            start=(c == 0), stop=(c == NCH - 1),
                            tile_position=(0, 32 * t),
                        )
                if pair == 0 and pending_tails:
                    pending_tails.pop(0)()
                if rep == repeat - 1 and p == NPACK - 1 and pair == 0:
                    # early logits for units 0-5 + first AllGather (hidden under pack-2 compute)
                    h1_ps = tpps.tile([6, 64], F32, tag="tp", name="h1_ps")
                    gt0 = guard(nc.tensor, [d_fc1, d_fc1b, d_fc2])
                    mmh1 = nc.tensor.matmul(h1_ps[:], biT_sb[:, 0:6], fc1T_sb[:], start=True, stop=True)
                    pin(mmh1, gt0)
                    hb1 = tailp.tile([6, 64], F32, name="hb1")
                    nc.vector.tensor_add(hb1[:], h1_ps[:], fc1b_sb[0:6, :])
                    hth1 = tailp.tile([6, 64], F32, name="hth1")
                    nc.scalar.activation(hth1[:], hb1[:], AF.Tanh)
                    hT1_ps = tpps.tile([64, 6], F32, tag="tp", name="hT1_ps")
                    nc.tensor.transpose(hT1_ps[:], hth1[:], ident[0:6, 0:6])
                    hT1 = tailp.tile([64, 6], F32, name="hT1")
                    nc.vector.tensor_copy(hT1[:], hT1_ps[:])
                    ci1_ps = tpps.tile([6, 1], F32, tag="tp", name="ci1_ps")
                    nc.tensor.matmul(ci1_ps[:], hT1[:], fc2T_sb[:], start=True, stop=True)
                    eci1 = tailp.tile([6, 1], F32, name="eci1")
                    nc.scalar.activation(eci1[:], ci1_ps[:], AF.Exp)
                    cc1_in = dramp.tile([6, 1], F32, name="cc1_in")
                    cc1_out = dramp.tile([NCORES * 6, 1], F32, name="cc1_out")
                    nc.sync.dma_start(cc1_in[:], eci1[:])
                    nc.gpsimd.collective_compute(
                        "AllGather", mybir.AluOpType.bypass,
                        replica_groups=[list(range(NCORES))],
                        ins=[cc1_in.opt()], outs=[cc1_out.opt()],
                    )
                    zl1 = tailp.tile([3, 16], F32, name="zl1")
                    nc.sync.dma_start(zl1[:], bass.AP(cc1_out[:].tensor, 0, [[1, 3], [3, 16]]))
                    zk1 = tailp.tile([3, 1], F32, name="zk1")
                    nc.vector.reduce_sum(zk1[:], zl1[:], axis=mybir.AxisListType.X)

            def make_tail(rep=rep, p=p):
                # ---- pack tail: w1 columns via transposes, o1 matmuls, Bi rows
                e0cs, csrows, o_ps_l, s0row = packdata[(rep, p)]
                o_ps = o_ps_l[0]
                nc.scalar.activation(e0cs[:], s0row[:], AF.Exp)
                crec = packp.tile([128, 4 * NCH], F32, tag="crec", name=f"crec_{rep}_{p}")
                ucols = packp.tile([128, 4 * NCH], F32, tag="ucols", name=f"ucols_{rep}_{p}")
                for ck in range(NCH):
                    tp_ps = tpps.tile([128, 4], F32, tag="tp", name=f"tpc_{rep}_{p}_{ck}")
                    nc.tensor.transpose(tp_ps[:], csrows[:, 128 * ck:128 * (ck + 1)], ident[:, 0:97:32])
                    nc.vector.reciprocal(crec[:, 4 * ck:4 * (ck + 1)], tp_ps[:, 0:4])
                for ck in range(NCH):
                    tp_ps = tpps.tile([128, 4], F32, tag="tp", name=f"tpe_{rep}_{p}_{ck}")
                    nc.tensor.transpose(tp_ps[:], e0cs[:, 128 * ck:128 * (ck + 1)], ident[:, 0:97:32])
                    nc.vector.tensor_mul(
                        ucols[:, 4 * ck:4 * (ck + 1)],
                        tp_ps[:, 0:4],
                        crec[:, 4 * ck:4 * (ck + 1)],
                    )
                for t in range(4):
                    for c in range(NCH):
                        nc.tensor.matmul(
                            o_ps[32 * t:32 * t + 1, 0:32],
                            ucols[:, 4 * c + t:4 * c + t + 1],
                            nb[p][:, 512 * t + 256 + 32 * c:512 * t + 256 + 32 * (c + 1)],
                            start=(c == 0), stop=(c == NCH - 1),
                            tile_position=(0, 32 * t),
                        )
                bi_rows = packp.tile([128, 64], F32, tag="bi", name=f"bi_{rep}_{p}")
                nc.vector.tensor_mul(bi_rows[:], o_ps[:], fg[p][:])
                tpb_ps = tpps.tile([128, 128], F32, tag="tp", name=f"tpb_{rep}_{p}")
                tpb = nc.tensor.transpose(tpb_ps[0:64, :], bi_rows[:, 0:64], ident[:])
                nc.vector.tensor_copy(biT_sb[:, 4 * p:4 * (p + 1)], tpb_ps[0:64, 0:97:32])
            pending_tails.append(make_tail)

        for _t in pending_tails:
            _t()
        pending_tails.clear()

        # ---------------- tail: FC + batch softmax + broadcast write ----------------
        gt1 = guard(nc.tensor, [d_fc1, d_fc1b, d_fc2, d_sel, d_oh])
        h_ps = tpps.tile([NU, 64], F32, tag="tp")
        mmh = nc.tensor.matmul(h_ps[:], biT_sb[:], fc1T_sb[:], start=True, stop=True)
        pin(mmh, gt1)
        hb = tailp.tile([NU, 64], F32)
        nc.vector.tensor_add(hb[:], h_ps[:], fc1b_sb[:])
        hth = tailp.tile([NU, 64], F32)
        nc.scalar.activation(hth[:], hb[:], AF.Tanh)
        hT_ps = ops.tile([64, NU], F32, tag="o")
        nc.tensor.transpose(hT_ps[:], hth[:], ident[0:12, 0:12])
        hT = tailp.tile([64, NU], F32)
        nc.vector.tensor_copy(hT[:], hT_ps[:])
        ci_ps = tpps.tile([NU, 1], F32, tag="tp")
        nc.tensor.matmul(ci_ps[:], hT[:], fc2T_sb[:], start=True, stop=True)
        eci = tailp.tile([NU, 1], F32)
        nc.scalar.activation(eci[:], ci_ps[:], AF.Exp)
        if _DEBUG:
            nc.sync.dma_start(dbg_eci[:, :], eci[:])

        # second AllGather: units 6-11 only; Z = zk1 + zk2
        cc2_in = dramp.tile([6, 1], F32, name="cc2_in")
        cc2_out = dramp.tile([NCORES * 6, 1], F32, name="cc2_out")
        nc.sync.dma_start(cc2_in[:], eci[6:12, :])
        nc.gpsimd.collective_compute(
            "AllGather",
            mybir.AluOpType.bypass,
            replica_groups=[list(range(NCORES))],
            ins=[cc2_in.opt()],
            outs=[cc2_out.opt()],
        )
        zl2 = tailp.tile([3, 16], F32, name="zl2")
        nc.sync.dma_start(zl2[:], bass.AP(cc2_out[:].tensor, 0, [[1, 3], [3, 16]]))
        zk2 = tailp.tile([3, 1], F32, name="zk2")
        nc.vector.reduce_sum(zk2[:], zl2[:], axis=mybir.AxisListType.X)
        zk = tailp.tile([3, 1], F32, name="zk")
        nc.vector.tensor_add(zk[:], zk1[:], zk2[:])
        # zcol[r] = Z_{r%3} via constant selection matmul
        zcol_ps = tpps.tile([NU, 1], F32, tag="tp", name="zcol_ps")
        gks = guard(nc.tensor, [d_ksel])
        mmz = nc.tensor.matmul(zcol_ps[:], ksel3[:], zk[:], start=True, stop=True)
        pin(mmz, gks)
        zr = tailp.tile([NU, 1], F32)
        nc.vector.reciprocal(zr[:], zcol_ps[:])
        alpha = tailp.tile([NU, 1], F32)
        nc.vector.tensor_mul(alpha[:], eci[:], zr[:])
        # selA = selT * alpha (per-partition scalar)
        selA = tailp.tile([NU, BPC], F32)
        nc.vector.tensor_scalar_mul(selA[:], selT[:], alpha[:])
        # Bi rows [12, 64] = transpose(biT)
        bi12_ps = ops.tile([NU, 64], F32, tag="o")
        nc.tensor.transpose(bi12_ps[:], biT_sb[:], ident[0:64, 0:64])
        bi12 = tailp.tile([NU, 64], F32)
        nc.vector.tensor_copy(bi12[:], bi12_ps[:])
        if _DEBUG:
            nc.sync.dma_start(dbg_bi[:, :], bi12[:])
        # rows[b] = sum_k alpha * Bi
        rows_ps = tpps.tile([BPC, 64], F32, tag="tp")
        nc.tensor.matmul(rows_ps[:], selA[:], bi12[:], start=True, stop=True)
        rows_sb = tailp.tile([BPC, 64], F32)
        nc.vector.tensor_copy(rows_sb[:], rows_ps[:])
        if _DEBUG:
            nc.sync.dma_start(dbg_rows[:, :], rows_sb[:])
        rep = tailp.tile([BPC, 512], F32)
        nc.vector.tensor_copy(
            rep[:].rearrange("p (r d) -> p r d", r=8),
            rows_sb[:, None, :].broadcast_to([BPC, 8, 64]),
        )
        for b in range(BPC):
            bc_ps = csps.tile([128, 512], F32, tag="cs", name=f"bc_ps{b}")
            nc.tensor.matmul(bc_ps[:], oh[:, 128 * b:128 * (b + 1)], rep[:], start=True, stop=True)
            bc_sb = tailp.tile([128, 512], F32, tag=f"bc{b % 2}")
            nc.vector.tensor_copy(bc_sb[:], bc_ps[:])
            nc.sync.dma_start(
                out[b].rearrange("(p r) d -> p (r d)", p=128),
                bc_sb[:],
            )
        ctx.close()
    nc.finalize()
    return nc


def make_in_maps(a_emb, v_emb, l_emb, fc1_w, fc1_b, fc2_w):
    embs = [a_emb, v_emb, l_emb]
    fc1T = np.ascontiguousarray(fc1_w.T, np.float32)           # [in, out]
    fc1b = np.ascontiguousarray(np.tile(fc1_b[None, :], (NU, 1)), np.float32)
    fc2T = np.ascontiguousarray(fc2_w.T, np.float32)           # [64, 1]
    in_maps = []
    for core in range(NCORES):
        eblob = np.zeros((NPACK, 128, 8192), ml_dtypes.bfloat16)
        nblob = np.zeros((NPACK, 128, 2048), np.float32)
        f0g0 = np.zeros((NPACK, 128, 64), np.float32)
        for u in range(NU):
            p, t = u // 4, u % 4
            b = BPC * core + u // 3
            fi, gi = PAIRS[u % 3]
            f = embs[fi][b]  # [1024, 32]
            g = embs[gi][b]
            fT32, gT32 = f.T.astype(np.float32), g.T.astype(np.float32)
            fhi = fT32.astype(ml_dtypes.bfloat16)
            ghi = gT32.astype(ml_dtypes.bfloat16)
            rb, off = 32 * (t % 2), 4096 * (t // 2)
            eblob[p, rb:rb + 32, off:off + 1024] = fhi
            eblob[p, rb:rb + 32, off + 1024:off + 2048] = ghi
            eblob[p, rb:rb + 32, off + 2048:off + 3072] = (fT32 - fhi.astype(np.float32)).astype(ml_dtypes.bfloat16)
            eblob[p, rb:rb + 32, off + 3072:off + 4096] = (gT32 - ghi.astype(np.float32)).astype(ml_dtypes.bfloat16)
            # natural chunked: [128, 256] with chunk c at cols 32c..32c+32
            fN = f.reshape(NCH, 128, D).transpose(1, 0, 2).reshape(128, NCH * D)
            gN = g.reshape(NCH, 128, D).transpose(1, 0, 2).reshape(128, NCH * D)
            nblob[p, :, 512 * t:512 * t + 256] = fN
            nblob[p, :, 512 * t + 256:512 * t + 512] = gN
            f0g0[p, 32 * t, 0:32] = f[0]
            f0g0[p, 32 * t, 32:64] = g[0]
        in_maps.append({
            "eblob": eblob, "nblob": nblob, "f0g0": f0g0,
            "fc1T": fc1T, "fc1b": fc1b, "fc2T": fc2T,
        })
    return in_maps


_PROGRAM_CACHE = {}


def _get_program(repeat=1):
    key = ("nc", repeat)
    if key not in _PROGRAM_CACHE:
        _PROGRAM_CACHE[key] = build_program(repeat)
    return _PROGRAM_CACHE[key]


def kernel(a_emb, v_emb, l_emb, fc1_w, fc1_b, fc2_w, _want_results=False):
    a_emb = np.asarray(a_emb, np.float32)
    v_emb = np.asarray(v_emb, np.float32)
    l_emb = np.asarray(l_emb, np.float32)
    fc1_w = np.asarray(fc1_w, np.float32)
    fc1_b = np.asarray(fc1_b, np.float32)
    fc2_w = np.asarray(fc2_w, np.float32)
    nc = _get_program()
    in_maps = make_in_maps(a_emb, v_emb, l_emb, fc1_w, fc1_b, fc2_w)
    res = None
    for attempt in range(3):
        try:
            res = run_bass_kernel_spmd(nc, in_maps, core_ids=list(range(NCORES)))
            break
        except Exception:
            if attempt == 2:
                raise
    assert res is not None
    outp = np.concatenate([res.results[c]["out"] for c in range(NCORES)], axis=0)
    if _want_results:
        return outp, res
    return outp



# revision 8
# speedup vs baseline: 1.1670x; 1.1670x over previous
"""Trainium2 Bass kernel for nn_AttnModel (BiAttn x3 + tiny FC + batch-softmax tile).

Contract: kernel(**inputs) takes the FULL inputs (a_emb/v_emb/l_emb [32,1024,32],
fc1_w [64,64], fc1_b [64], fc2_w [1,64]) and returns the FULL output [32,1024,64].

Strategy (8 NeuronCores, data-parallel over batch, 4 batches/core):
  Per (batch, pair) "unit" (pairs: (a,v), (a,l), (v,l)) the reference only uses
  row 0 of each BiAttn output, which needs:
    S = f @ g^T  [1024,1024]      (PE, K=32 row-tiled, one unit per 32-row tile)
    E = exp(S)                    (ScalarE, fused row-sum via accum_out)
    colsum c_j = sum_i E_ij       (PE ones-matmul, M=1, col-tiled 4 units)
    w1_j = E_0j / c_j ; o1 = w1 @ g
    w2_i = E_i0 / rowsum_i ; o2 = w2 @ f
    Bi_row = [o1*f_0, o2*g_0]  (64)
  Tiny FC -> logits Ci [4,3]; exp(Ci) AllGathered across the 8 cores for the
  batch-dim softmax; output row per batch broadcast-written as [1024, 64].
"""
import numpy as np
import ml_dtypes

import concourse.bass as bass
import concourse.bacc as bacc
import concourse.tile as tile
import concourse.mybir as mybir
from concourse.bass_utils import run_bass_kernel_spmd
from concourse.tile_rust import add_dep_helper

F32 = mybir.dt.float32
F32R = mybir.dt.float32r
BF16 = mybir.dt.bfloat16
AF = mybir.ActivationFunctionType

B, U, D = 32, 1024, 32
NCORES = 8
BPC = B // NCORES          # batches per core = 4
NU = 3 * BPC               # units per core = 12
NPACK = NU // 4            # packs of 4 units = 3
NCH = U // 128             # i-chunks = 8
PAIRS = [(0, 1), (0, 2), (1, 2)]  # (f,g) emb indices for pair k; 0=a 1=v 2=l

_DEBUG = False
import os as _os
S_F32R = _os.environ.get("S_F32R", "0") == "1"


def build_program(repeat=1):
    nc = bacc.Bacc("TRN2", target_bir_lowering=False, debug=False, num_devices=NCORES)

    eblob = nc.dram_tensor("eblob", [NPACK, 128, 2048], F32R, kind="ExternalInput")
    nblob = nc.dram_tensor("nblob", [NPACK, 128, 2048], F32, kind="ExternalInput")
    f0g0 = nc.dram_tensor("f0g0", [NPACK, 128, 64], F32, kind="ExternalInput")
    fc1T = nc.dram_tensor("fc1T", [64, 64], F32, kind="ExternalInput")
    fc1b = nc.dram_tensor("fc1b", [NU, 64], F32, kind="ExternalInput")
    fc2T = nc.dram_tensor("fc2T", [64, 1], F32, kind="ExternalInput")
    out = nc.dram_tensor("out", [BPC, U, 2 * D], F32, kind="ExternalOutput")
    if _DEBUG:
        dbg_bi = nc.dram_tensor("dbg_bi", [NU, 64], F32, kind="ExternalOutput")
        dbg_eci = nc.dram_tensor("dbg_eci", [NU, 1], F32, kind="ExternalOutput")
        dbg_rows = nc.dram_tensor("dbg_rows", [BPC, 64], F32, kind="ExternalOutput")

    ident_np = np.eye(128, dtype=np.float32)
    sel_np = np.zeros((NU, BPC), np.float32)
    for r in range(NU):
        sel_np[r, r // 3] = 1.0
    oh_np = np.zeros((BPC, BPC * 128), np.float32)
    for b in range(BPC):
        oh_np[b, 128 * b:128 * (b + 1)] = 1.0

    with tile.TileContext(nc) as tc:
        from contextlib import ExitStack
        ctx = ExitStack()
        consts = ctx.enter_context(tc.tile_pool(name="consts", bufs=1))
        bigp = ctx.enter_context(tc.tile_pool(name="big", bufs=1))
        epool = ctx.enter_context(tc.tile_pool(name="epool", bufs=1))
        packp = ctx.enter_context(tc.tile_pool(name="packp", bufs=3))
        tailp = ctx.enter_context(tc.tile_pool(name="tailp", bufs=1))
        dramp = ctx.enter_context(tc.tile_pool(name="dramp", bufs=1, space="DRAM"))

        sps = ctx.enter_context(tc.tile_pool(name="sps", bufs=1, space="PSUM"))
        csps = ctx.enter_context(tc.tile_pool(name="csps", bufs=1, space="PSUM"))
        tpps = ctx.enter_context(tc.tile_pool(name="tpps", bufs=1, space="PSUM"))
        ops = ctx.enter_context(tc.tile_pool(name="ops", bufs=1, space="PSUM"))

        # ---------------- first-needed input: pack 0 (fp32 fT/gT, 4 units) ----------------
        ebt = []
        d_eb = []
        t_e0 = bigp.tile([128, 2048], F32R, tag="ebA0", name="eb_0")
        d_eb.append(nc.sync.dma_start(t_e0[:], eblob[0, :, :]))
        ebt.append(t_e0)

        # ---------------- constants ----------------
        ident = consts.tile([128, 128], F32)
        d_ident = nc.sync.dma_start(ident[:], nc.inline_tensor(ident_np, name="c_ident")[:, :])
        ones = consts.tile([128, 1], F32)
        d_ones = nc.sync.dma_start(ones[:], nc.inline_tensor(np.ones((128, 1), np.float32), name="c_ones")[:, :])
        ones_r = consts.tile([128, 1], F32)
        d_ones_r = nc.sync.dma_start(ones_r[:].bitcast(F32R), nc.inline_tensor(np.ones((128, 1), np.float32), name="c_ones_r")[:, :].bitcast(F32R))
        selT = consts.tile([NU, BPC], F32)
        d_sel = nc.sync.dma_start(selT[:], nc.inline_tensor(sel_np, name="c_sel")[:, :])
        oh = consts.tile([BPC, BPC * 128], F32)
        d_oh = nc.sync.dma_start(oh[:], nc.inline_tensor(oh_np, name="c_oh")[:, :])
        fc1T_sb = consts.tile([64, 64], F32)
        d_fc1 = nc.sync.dma_start(fc1T_sb[:], fc1T[:, :])
        fc1b_sb = consts.tile([NU, 64], F32)
        d_fc1b = nc.sync.dma_start(fc1b_sb[:], fc1b[:, :])
        fc2T_sb = consts.tile([64, 1], F32)
        d_fc2 = nc.sync.dma_start(fc2T_sb[:], fc2T[:, :])
        ksel_np = np.zeros((3, NU), np.float32)
        for r in range(NU):
            ksel_np[r % 3, r] = 1.0
        ksel3 = consts.tile([3, NU], F32)
        d_ksel = nc.sync.dma_start(ksel3[:], nc.inline_tensor(ksel_np, name="c_ksel")[:, :])

        # ---------------- resident inputs ----------------
        nb, fg = [], []
        d_nb, d_fg = [], []
        for p in range(NPACK):
            if p > 0:
                t_eA = bigp.tile([128, 2048], F32R, tag=f"ebA{p}", name=f"eb_{p}")
                d_eb.append(nc.sync.dma_start(t_eA[:], eblob[p, :, :]))
                ebt.append(t_eA)
            t_n = bigp.tile([128, 2048], F32, tag="nb", bufs=2, name=f"nb_{p}")
            d_nb.append(nc.sync.dma_start(t_n[:], nblob[p, :, :]))
            nb.append(t_n)
            t_f = bigp.tile([128, 64], F32, tag=f"fg{p}")
            d_fg.append(nc.sync.dma_start(t_f[:], f0g0[p, :, :]))
            fg.append(t_f)

        def guard(eng, deps):
            deps = [d for d in deps if d is not None]
            if not deps:
                return None
            n = eng.nop(nofuse=True)
            for d in deps:
                add_dep_helper(n.ins, d.ins, sync=True, reason="wait-carrier")
            return n

        def pin(inst, g):
            if g is not None:
                add_dep_helper(inst.ins, g.ins, sync=False, reason="order")

        biT_sb = tailp.tile([64, NU], F32)

        # ---------------- main loop: one unit at a time, full-unit E ----------------
        SR = F32R if S_F32R else F32
        def maybe_r(ap):
            return ap.bitcast(F32R) if S_F32R else ap

        e_readers = {}   # unit -> DVE reader insts of its E tile
        packdata = {}
        pending_tails = []
        for rep, p in [(r_, p_) for r_ in range(repeat) for p_ in range(NPACK)]:
            e0cs = packp.tile([128, 1024], F32, tag="e0cs", name=f"e0cs_{rep}_{p}")
            s0row = packp.tile([128, 1024], F32, tag="s0row", name=f"s0row_{rep}_{p}")
            csrows = packp.tile([128, 1024], F32, tag="csrows", name=f"csrows_{rep}_{p}")
            nc.gpsimd.memset(s0row[:], 0.0)
            packdata[(rep, p)] = (e0cs, csrows, [], s0row)
            for pair in range(2):
                tpair = (2 * pair, 2 * pair + 1)
                ud = {}
                for t in tpair:
                    u = 4 * (p + NPACK * rep) + t
                    E = epool.tile([128, NCH * 1024], F32, tag=f"E{u % 2}", name=f"E_{u}")
                    rs = packp.tile([128, NCH], F32, tag=f"rs{t}", name=f"rs_{u}")
                    scol0 = packp.tile([128, NCH], F32, tag=f"scol0{t}", name=f"scol0_{u}")
                    g0 = guard(nc.tensor, [d_eb[p]] if t % 2 == 0 else [])
                    ge = guard(nc.scalar, e_readers.get(u - 2, []))
                    ud[t] = (u, E, rs, scol0, [], g0, ge)
                # chunk-major over the pair: row tiles 32*t0 / 32*t1 stream concurrently
                for c in range(NCH):
                    for t in tpair:
                        u, E, rs, scol0, s_readers, g0, ge = ud[t]
                        S_ps = sps.tile([128, 1024], F32, tag=f"S{u % 2}", name=f"S_{u}_{c}")
                        rb = 32 * t
                        eslc = ebt[p]
                        fch = eslc[rb:rb + 32, 128 * c:128 * (c + 1)]
                        for h in range(2):
                            gh = eslc[rb:rb + 32, 1024 + 512 * h:1024 + 512 * (h + 1)]
                            out_h = S_ps[:, 512 * h:512 * (h + 1)]
                            mm = nc.tensor.matmul(out_h, fch, gh, start=True, stop=True,
                                                  tile_position=(rb, 0))
                            if c == 0 and h == 0:
                                pin(mm, g0)
                        act = nc.scalar.activation(
                            E[:, 1024 * c:1024 * (c + 1)].bitcast(F32R), S_ps[:], AF.Exp,
                            accum_out=rs[:, c:c + 1])
                        if c == 0:
                            pin(act, ge)
                            rsp = nc.vector.tensor_copy(s0row[32 * t:32 * t + 1, :], S_ps[0:1, :])
                        csp = nc.vector.tensor_copy(scol0[:, c:c + 1], S_ps[:, 0:1])
                        s_readers.extend([rsp if c == 0 else None, csp])
                # pair tail: colsums + w2 + o2 per unit
                for t in tpair:
                    u, E, rs, scol0, s_readers, g0, ge = ud[t]
                    cs_ps = csps.tile([1, 1024], F32, tag="cs", name=f"cs_{u}")
                    for c in range(NCH):
                        for h in range(2):
                            nc.tensor.matmul(
                                cs_ps[0:1, 512 * h:512 * (h + 1)],
                                ones_r[:, :].bitcast(F32R),
                                E[:, 1024 * c + 512 * h:1024 * c + 512 * (h + 1)].bitcast(F32R),
                                start=(c == 0), stop=(c == NCH - 1),
                            )
                    r_cs = nc.vector.tensor_copy(csrows[32 * t:32 * t + 1, :], cs_ps[0:1, :])
                    ec0 = packp.tile([128, NCH], F32, tag=f"ec0{t}", name=f"ec0_{u}")
                    nc.scalar.activation(ec0[:], scol0[:], AF.Exp)
                    e_readers[u] = [r_cs]
                    rsr = packp.tile([128, NCH], F32, tag=f"rsr{t}", name=f"rsr_{u}")
                    nc.vector.reciprocal(rsr[:], rs[:])
                    w2 = packp.tile([128, NCH], F32, tag=f"w2{t}", name=f"w2_{u}")
                    nc.vector.tensor_mul(w2[:], ec0[:], rsr[:])
                    o_ps = packdata[(rep, p)][2]
                    if t == 0:
                        o_ps.append(ops.tile([128, 64], F32, tag="o", name=f"o_ps_{rep}_{p}"))
                    for c in range(NCH):
                        nc.tensor.matmul(
                            o_ps[0][32 * t:32 * t + 1, 32:64],
                            w2[:, c:c + 1],
                            nb[p][:, 512 * t + 32 * c:512 * t + 32 * (c + 1)],
                            start=(c == 0), stop=(c == NCH - 1),
                            tile_position=(0, 32 * t),
                        )
                if pair == 0 and pending_tails:
                    pending_tails.pop(0)()
                if rep == repeat - 1 and p == NPACK - 1 and pair == 0:
                    # early logits for units 0-5 + first AllGather (hidden under pack-2 compute)
                    h1_ps = tpps.tile([6, 64], F32, tag="tp", name="h1_ps")
                    gt0 = guard(nc.tensor, [d_fc1, d_fc1b, d_fc2])
                    mmh1 = nc.tensor.matmul(h1_ps[:], biT_sb[:, 0:6], fc1T_sb[:], start=True, stop=True)
                    pin(mmh1, gt0)
                    hb1 = tailp.tile([6, 64], F32, name="hb1")
                    nc.vector.tensor_add(hb1[:], h1_ps[:], fc1b_sb[0:6, :])
                    hth1 = tailp.tile([6, 64], F32, name="hth1")
                    nc.scalar.activation(hth1[:], hb1[:], AF.Tanh)
                    hT1_ps = tpps.tile([64, 6], F32, tag="tp", name="hT1_ps")
                    nc.tensor.transpose(hT1_ps[:], hth1[:], ident[0:6, 0:6])
                    hT1 = tailp.tile([64, 6], F32, name="hT1")
                    nc.vector.tensor_copy(hT1[:], hT1_ps[:])
                    ci1_ps = tpps.tile([6, 1], F32, tag="tp", name="ci1_ps")
                    nc.tensor.matmul(ci1_ps[:], hT1[:], fc2T_sb[:], start=True, stop=True)
                    eci1 = tailp.tile([6, 1], F32, name="eci1")
                    nc.scalar.activation(eci1[:], ci1_ps[:], AF.Exp)
                    cc1_in = dramp.tile([6, 1], F32, name="cc1_in")
                    cc1_out = dramp.tile([NCORES * 6, 1], F32, name="cc1_out")
                    nc.sync.dma_start(cc1_in[:], eci1[:])
                    nc.gpsimd.collective_compute(
                        "AllGather", mybir.AluOpType.bypass,
                        replica_groups=[list(range(NCORES))],
                        ins=[cc1_in.opt()], outs=[cc1_out.opt()],
                    )
                    zl1 = tailp.tile([3, 16], F32, name="zl1")
                    nc.sync.dma_start(zl1[:], bass.AP(cc1_out[:].tensor, 0, [[1, 3], [3, 16]]))
                    zk1 = tailp.tile([3, 1], F32, name="zk1")
                    nc.vector.reduce_sum(zk1[:], zl1[:], axis=mybir.AxisListType.X)

            def make_tail(rep=rep, p=p):
                # ---- pack tail: w1 columns via transposes, o1 matmuls, Bi rows
                e0cs, csrows, o_ps_l, s0row = packdata[(rep, p)]
                o_ps = o_ps_l[0]
                nc.scalar.activation(e0cs[:], s0row[:], AF.Exp)
                crec = packp.tile([128, 4 * NCH], F32, tag="crec", name=f"crec_{rep}_{p}")
                ucols = packp.tile([128, 4 * NCH], F32, tag="ucols", name=f"ucols_{rep}_{p}")
                for ck in range(NCH):
                    tp_ps = tpps.tile([128, 4], F32, tag="tp", name=f"tpc_{rep}_{p}_{ck}")
                    nc.tensor.transpose(tp_ps[:], csrows[:, 128 * ck:128 * (ck + 1)], ident[:, 0:97:32])
                    nc.vector.reciprocal(crec[:, 4 * ck:4 * (ck + 1)], tp_ps[:, 0:4])
                for ck in range(NCH):
                    tp_ps = tpps.tile([128, 4], F32, tag="tp", name=f"tpe_{rep}_{p}_{ck}")
                    nc.tensor.transpose(tp_ps[:], e0cs[:, 128 * ck:128 * (ck + 1)], ident[:, 0:97:32])
                    nc.vector.tensor_mul(
                        ucols[:, 4 * ck:4 * (ck + 1)],
                        tp_ps[:, 0:4],
                        crec[:, 4 * ck:4 * (ck + 1)],
                    )
                for t in range(4):
                    for c in range(NCH):
                        nc.tensor.matmul(
                            o_ps[32 * t:32 * t + 1, 0:32],
                            ucols[:, 4 * c + t:4 * c + t + 1],
                            nb[p][:, 512 * t + 256 + 32 * c:512 * t + 256 + 32 * (c + 1)],
                            start=(c == 0), stop=(c == NCH - 1),
                            tile_position=(0, 32 * t),
                        )
                bi_rows = packp.tile([128, 64], F32, tag="bi", name=f"bi_{rep}_{p}")
                nc.vector.tensor_mul(bi_rows[:], o_ps[:], fg[p][:])
                tpb_ps = tpps.tile([128, 128], F32, tag="tp", name=f"tpb_{rep}_{p}")
                tpb = nc.tensor.transpose(tpb_ps[0:64, :], bi_rows[:, 0:64], ident[:])
                nc.vector.tensor_copy(biT_sb[:, 4 * p:4 * (p + 1)], tpb_ps[0:64, 0:97:32])
            pending_tails.append(make_tail)

        for _t in pending_tails:
            _t()
        pending_tails.clear()

        # ---------------- tail: FC + batch softmax + broadcast write ----------------
        gt1 = guard(nc.tensor, [d_fc1, d_fc1b, d_fc2, d_sel, d_oh])
        h_ps = tpps.tile([NU, 64], F32, tag="tp")
        mmh = nc.tensor.matmul(h_ps[:], biT_sb[:], fc1T_sb[:], start=True, stop=True)
        pin(mmh, gt1)
        hb = tailp.tile([NU, 64], F32)
        nc.vector.tensor_add(hb[:], h_ps[:], fc1b_sb[:])
        hth = tailp.tile([NU, 64], F32)
        nc.scalar.activation(hth[:], hb[:], AF.Tanh)
        hT_ps = ops.tile([64, NU], F32, tag="o")
        nc.tensor.transpose(hT_ps[:], hth[:], ident[0:12, 0:12])
        hT = tailp.tile([64, NU], F32)
        nc.vector.tensor_copy(hT[:], hT_ps[:])
        ci_ps = tpps.tile([NU, 1], F32, tag="tp")
        nc.tensor.matmul(ci_ps[:], hT[:], fc2T_sb[:], start=True, stop=True)
        eci = tailp.tile([NU, 1], F32)
        nc.scalar.activation(eci[:], ci_ps[:], AF.Exp)
        if _DEBUG:
            nc.sync.dma_start(dbg_eci[:, :], eci[:])

        # second AllGather: units 6-11 only; Z = zk1 + zk2
        cc2_in = dramp.tile([6, 1], F32, name="cc2_in")
        cc2_out = dramp.tile([NCORES * 6, 1], F32, name="cc2_out")
        nc.sync.dma_start(cc2_in[:], eci[6:12, :])
        nc.gpsimd.collective_compute(
            "AllGather",
            mybir.AluOpType.bypass,
            replica_groups=[list(range(NCORES))],
            ins=[cc2_in.opt()],
            outs=[cc2_out.opt()],
        )
        zl2 = tailp.tile([3, 16], F32, name="zl2")
        nc.sync.dma_start(zl2[:], bass.AP(cc2_out[:].tensor, 0, [[1, 3], [3, 16]]))
        zk2 = tailp.tile([3, 1], F32, name="zk2")
        nc.vector.reduce_sum(zk2[:], zl2[:], axis=mybir.AxisListType.X)
        zk = tailp.tile([3, 1], F32, name="zk")
        nc.vector.tensor_add(zk[:], zk1[:], zk2[:])
        # zcol[r] = Z_{r%3} via constant selection matmul
        zcol_ps = tpps.tile([NU, 1], F32, tag="tp", name="zcol_ps")
        gks = guard(nc.tensor, [d_ksel])
        mmz = nc.tensor.matmul(zcol_ps[:], ksel3[:], zk[:], start=True, stop=True)
        pin(mmz, gks)
        zr = tailp.tile([NU, 1], F32)
        nc.vector.reciprocal(zr[:], zcol_ps[:])
        alpha = tailp.tile([NU, 1], F32)
        nc.vector.tensor_mul(alpha[:], eci[:], zr[:])
        # selA = selT * alpha (per-partition scalar)
        selA = tailp.tile([NU, BPC], F32)
        nc.vector.tensor_scalar_mul(selA[:], selT[:], alpha[:])
        # Bi rows [12, 64] = transpose(biT)
        bi12_ps = ops.tile([NU, 64], F32, tag="o")
        nc.tensor.transpose(bi12_ps[:], biT_sb[:], ident[0:64, 0:64])
        bi12 = tailp.tile([NU, 64], F32)
        nc.vector.tensor_copy(bi12[:], bi12_ps[:])
        if _DEBUG:
            nc.sync.dma_start(dbg_bi[:, :], bi12[:])
        # rows[b] = sum_k alpha * Bi
        rows_ps = tpps.tile([BPC, 64], F32, tag="tp")
        nc.tensor.matmul(rows_ps[:], selA[:], bi12[:], start=True, stop=True)
        rows_sb = tailp.tile([BPC, 64], F32)
        nc.vector.tensor_copy(rows_sb[:], rows_ps[:])
        if _DEBUG:
            nc.sync.dma_start(dbg_rows[:, :], rows_sb[:])
        rep = tailp.tile([BPC, 512], F32)
        nc.vector.tensor_copy(
            rep[:].rearrange("p (r d) -> p r d", r=8),
            rows_sb[:, None, :].broadcast_to([BPC, 8, 64]),
        )
        for b in range(BPC):
            bc_ps = csps.tile([128, 512], F32, tag="cs", name=f"bc_ps{b}")
            nc.tensor.matmul(bc_ps[:], oh[:, 128 * b:128 * (b + 1)], rep[:], start=True, stop=True)
            bc_sb = tailp.tile([128, 512], F32, tag=f"bc{b % 2}")
            nc.vector.tensor_copy(bc_sb[:], bc_ps[:])
            nc.sync.dma_start(
                out[b].rearrange("(p r) d -> p (r d)", p=128),
                bc_sb[:],
            )
        ctx.close()
    nc.finalize()
    return nc


def make_in_maps(a_emb, v_emb, l_emb, fc1_w, fc1_b, fc2_w):
    embs = [a_emb, v_emb, l_emb]
    fc1T = np.ascontiguousarray(fc1_w.T, np.float32)           # [in, out]
    fc1b = np.ascontiguousarray(np.tile(fc1_b[None, :], (NU, 1)), np.float32)
    fc2T = np.ascontiguousarray(fc2_w.T, np.float32)           # [64, 1]
    in_maps = []
    for core in range(NCORES):
        eblob = np.zeros((NPACK, 128, 2048), np.float32)
        nblob = np.zeros((NPACK, 128, 2048), np.float32)
        f0g0 = np.zeros((NPACK, 128, 64), np.float32)
        for u in range(NU):
            p, t = u // 4, u % 4
            b = BPC * core + u // 3
            fi, gi = PAIRS[u % 3]
            f = embs[fi][b]  # [1024, 32]
            g = embs[gi][b]
            fT32, gT32 = f.T.astype(np.float32), g.T.astype(np.float32)
            rb = 32 * t
            eblob[p, rb:rb + 32, 0:1024] = fT32
            eblob[p, rb:rb + 32, 1024:2048] = gT32
            # natural chunked: [128, 256] with chunk c at cols 32c..32c+32
            fN = f.reshape(NCH, 128, D).transpose(1, 0, 2).reshape(128, NCH * D)
            gN = g.reshape(NCH, 128, D).transpose(1, 0, 2).reshape(128, NCH * D)
            nblob[p, :, 512 * t:512 * t + 256] = fN
            nblob[p, :, 512 * t + 256:512 * t + 512] = gN
            f0g0[p, 32 * t, 0:32] = f[0]
            f0g0[p, 32 * t, 32:64] = g[0]
        in_maps.append({
            "eblob": eblob, "nblob": nblob, "f0g0": f0g0,
            "fc1T": fc1T, "fc1b": fc1b, "fc2T": fc2T,
        })
    return in_maps


_PROGRAM_CACHE = {}


def _get_program(repeat=1):
    key = ("nc", repeat)
    if key not in _PROGRAM_CACHE:
        _PROGRAM_CACHE[key] = build_program(repeat)
    return _PROGRAM_CACHE[key]


def kernel(a_emb, v_emb, l_emb, fc1_w, fc1_b, fc2_w, _want_results=False):
    a_emb = np.asarray(a_emb, np.float32)
    v_emb = np.asarray(v_emb, np.float32)
    l_emb = np.asarray(l_emb, np.float32)
    fc1_w = np.asarray(fc1_w, np.float32)
    fc1_b = np.asarray(fc1_b, np.float32)
    fc2_w = np.asarray(fc2_w, np.float32)
    nc = _get_program()
    in_maps = make_in_maps(a_emb, v_emb, l_emb, fc1_w, fc1_b, fc2_w)
    res = None
    for attempt in range(3):
        try:
            res = run_bass_kernel_spmd(nc, in_maps, core_ids=list(range(NCORES)))
            break
        except Exception:
            if attempt == 2:
                raise
    assert res is not None
    outp = np.concatenate([res.results[c]["out"] for c in range(NCORES)], axis=0)
    if _want_results:
        return outp, res
    return outp

